# revision 12
# baseline (speedup 1.0000x reference)
"""LocalL1Loss Trainium2 kernel (8 NeuronCores, pure data parallel).

Reference semantics (KERNEL_SIZE=7):
    tp = zero-pad(targets, 3 on each spatial side)
    d_s = mean_c |inputs - shift_s(tp)|      for the 49 shifts s
    out = mean_{n,h,w} min_s d_s

Per core (2 of 16 batch items):
  - host: cast to bf16; zero-pad targets; pre-arrange into the exact SBUF
    layouts (128 partitions = 16x8 grid of 32x64 pixel patches, 6
    (batch,channel) planes per partition; targets carry a 3px halo -> 38x70
    per patch, stored twice with a one-element column offset so every column
    shift reads a 4-byte-aligned bf16 window -> VectorE 2x mode throughout).
    Loads are a few large fully-contiguous DMAs.
  - per shift (both batch items in one wide op): VectorE subtract (bf16 2x),
    ScalarE Abs in place (parallel engine), VectorE 2 adds (channel sum) +
    running min.  Even-column shifts run first so the odd-copy DMAs hide
    behind compute (min is order-invariant).
  - epilogue: free-dim reduce_sum -> [128,1] fp32 partials -> DRAM; host sums
    8x128 partials and divides by 3*N*H*W.
"""

import numpy as np
import ml_dtypes

import concourse.bacc as bacc
import concourse.mybir as mybir
from concourse import tile
from concourse.bass_utils import run_bass_kernel_spmd

# geometry (hardcoded for the [16, 3, 512, 512] problem)
B, C, H, W = 16, 3, 512, 512
K = 7
PAD = K // 2
NCORES = 8
BC = B // NCORES            # batch per core = 2
PLANES = BC * C             # 6 (n, c) planes per core
PH, PW = 16, 8              # patch grid -> 128 partitions
PR, PC = H // PH, W // PW   # 32 x 64 patch
HR, HC = PR + K - 1, PC + K - 1          # 38 x 70 halo patch
HP, WP = H + K - 1, W + K - 1 + 1        # padded target: 518 x 519 (+1 col)

BF16 = mybir.dt.bfloat16
F32 = mybir.dt.float32

_CACHE = {}


def _build():
    nc = bacc.Bacc("TRN2", target_bir_lowering=False, debug=False,
                   num_devices=NCORES)

    x_d = nc.dram_tensor("x", [128, PLANES, PR, PC], BF16,
                         kind="ExternalInput")
    te_d = nc.dram_tensor("te", [128, PLANES, HR, HC], BF16,
                          kind="ExternalInput")
    to_d = nc.dram_tensor("to", [128, PLANES, HR, HC], BF16,
                          kind="ExternalInput")
    out_d = nc.dram_tensor("out", [128, 1], F32, kind="ExternalOutput")

    # even-j shifts first: the odd-copy (to) DMAs hide behind their compute
    shifts = [(i, j) for i in range(K) for j in range(K) if j % 2 == 0] + \
             [(i, j) for i in range(K) for j in range(K) if j % 2 == 1]

    with tile.TileContext(nc) as tc:
        with (
            tc.tile_pool(name="persist", bufs=1) as pp,
            tc.tile_pool(name="work", bufs=2) as wp,
        ):
            xt = pp.tile([128, PLANES, PR * PC], BF16, name="xt", tag="xt")
            te = pp.tile([128, PLANES, HR, HC], BF16, name="te", tag="te")
            to = pp.tile([128, PLANES, HR, HC], BF16, name="to", tag="to")
            best = pp.tile([128, BC, PR * PC], BF16, name="best", tag="best")

            # loads: one large contiguous DMA per (tensor, plane)
            for p in range(PLANES):
                nc.sync.dma_start(out=xt[:, p], in_=x_d.ap()[:, p])
            for p in range(PLANES):
                nc.sync.dma_start(out=te[:, p], in_=te_d.ap()[:, p])
            for p in range(PLANES):
                nc.sync.dma_start(out=to[:, p], in_=to_d.ap()[:, p])

            for si, (i, j) in enumerate(shifts):
                tsel, joff = (te, j) if j % 2 == 0 else (to, j - 1)
                diff = wp.tile([128, PLANES, PR, PC], BF16, tag="diff")
                nc.vector.tensor_tensor(
                    out=diff[:],
                    in0=xt[:].rearrange("p n (r w) -> p n r w", r=PR),
                    in1=tsel[:, :, i:i + PR, joff:joff + PC],
                    op=mybir.AluOpType.subtract)
                nc.scalar.activation(        # in-place |.| on ScalarE
                    out=diff[:], in_=diff[:],
                    func=mybir.ActivationFunctionType.Abs)
                a2 = diff[:].rearrange("p (n c) r w -> p n c (r w)", c=C)
                s1 = wp.tile([128, BC, PR * PC], BF16, tag="s1")
                nc.vector.tensor_tensor(
                    out=s1[:], in0=a2[:, :, 0], in1=a2[:, :, 1],
                    op=mybir.AluOpType.add)
                if si == 0:
                    nc.vector.tensor_tensor(
                        out=best[:], in0=s1[:], in1=a2[:, :, 2],
                        op=mybir.AluOpType.add)
                else:
                    s2 = wp.tile([128, BC, PR * PC], BF16, tag="s2")
                    nc.vector.tensor_tensor(
                        out=s2[:], in0=s1[:], in1=a2[:, :, 2],
                        op=mybir.AluOpType.add)
                    nc.vector.tensor_tensor(
                        out=best[:], in0=best[:], in1=s2[:],
                        op=mybir.AluOpType.min)

            # epilogue: per-partition partial sums
            rsum = pp.tile([128, 1], F32, name="rsum", tag="rsum")
            nc.vector.tensor_reduce(
                out=rsum[:], in_=best[:], axis=mybir.AxisListType.XY,
                op=mybir.AluOpType.add)
            nc.sync.dma_start(out=out_d.ap(), in_=rsum[:])

    nc.compile()
    return nc


def _prep(inputs, targets):
    bf = ml_dtypes.bfloat16
    inputs = np.asarray(inputs, dtype=np.float32)
    targets = np.asarray(targets, dtype=np.float32)
    x_bf = inputs.astype(bf)
    tp = np.zeros((B, C, HP, WP), dtype=np.float32)
    tp[:, :, PAD:PAD + H, PAD:PAD + W] = targets
    t_bf = tp.astype(bf)

    def halo(base):                      # base: [BC, C, HP, >=WP-1] bf16 view
        s = base.strides
        v = np.lib.stride_tricks.as_strided(
            base, shape=(BC, C, PH, PW, HR, HC),
            strides=(s[0], s[1], PR * s[2], PC * s[3], s[2], s[3]))
        return np.ascontiguousarray(v.transpose(2, 3, 0, 1, 4, 5)
                                    .reshape(128, PLANES, HR, HC))

    in_maps = []
    for core in range(NCORES):
        sl = slice(core * BC, (core + 1) * BC)
        xs = (x_bf[sl].reshape(BC, C, PH, PR, PW, PC)
              .transpose(2, 4, 0, 1, 3, 5).reshape(128, PLANES, PR, PC))
        in_maps.append({
            "x": np.ascontiguousarray(xs),
            "te": halo(t_bf[sl]),
            "to": halo(t_bf[sl, :, :, 1:]),
        })
    return in_maps


def _run(inputs, targets, trace=False, **kw):
    if "nc" not in _CACHE:
        _CACHE["nc"] = _build()
    nc = _CACHE["nc"]
    in_maps = _prep(inputs, targets)
    res = run_bass_kernel_spmd(nc, in_maps, list(range(NCORES)),
                               trace=trace, **kw)
    total = 0.0
    for core in range(NCORES):
        total += res.results[core]["out"].astype(np.float64).sum()
    val = np.float32(total / (C * B * H * W))
    return np.asarray(val, dtype=np.float32), res


def kernel(inputs, targets):
    out, _ = _run(inputs, targets)
    return out


# revision 13
# speedup vs baseline: 1.0203x; 1.0203x over previous
"""LocalL1Loss Trainium2 kernel (8 NeuronCores, pure data parallel).

Reference semantics (KERNEL_SIZE=7):
    tp = zero-pad(targets, 3 on each spatial side)
    d_s = mean_c |inputs - shift_s(tp)|      for the 49 shifts s
    out = mean_{n,h,w} min_s d_s

Per core (2 of 16 batch items):
  - host: cast to bf16; zero-pad targets; pre-arrange into the exact SBUF
    layouts (128 partitions = 16x8 grid of 32x64 pixel patches, 3 channel
    planes per batch item per partition; targets carry a 3px halo -> 38x70
    per patch, stored twice with a one-element column offset so every column
    shift reads a 4-byte-aligned bf16 window -> VectorE 2x mode throughout).
    Loads are a few large fully-contiguous DMAs.
  - per shift: VectorE subtract (bf16 2x), ScalarE Abs (parallel engine),
    VectorE 2 adds (channel sum) + running min.  Shifts are ordered so
    compute starts as soon as the first batch item's aligned-copy targets
    land; the remaining DMAs hide behind compute (min is order-invariant).
  - epilogue: free-dim reduce_sum -> [128,1] fp32 partials -> DRAM; host sums
    8x128 partials and divides by 3*N*H*W.
"""

import numpy as np
import ml_dtypes

import concourse.bacc as bacc
import concourse.mybir as mybir
from concourse import tile
from concourse.bass_utils import run_bass_kernel_spmd

# geometry (hardcoded for the [16, 3, 512, 512] problem)
B, C, H, W = 16, 3, 512, 512
K = 7
PAD = K // 2
NCORES = 8
BC = B // NCORES            # batch per core = 2
PH, PW = 16, 8              # patch grid -> 128 partitions
PR, PC = H // PH, W // PW   # 32 x 64 patch
HR, HC = PR + K - 1, PC + K - 1          # 38 x 70 halo patch
HP, WP = H + K - 1, W + K - 1 + 1        # padded target: 518 x 519 (+1 col)

BF16 = mybir.dt.bfloat16
F32 = mybir.dt.float32

_CACHE = {}


def _build():
    nc = bacc.Bacc("TRN2", target_bir_lowering=False, debug=False,
                   num_devices=NCORES)

    x_d = [nc.dram_tensor(f"x{n}", [128, C, PR, PC], BF16,
                          kind="ExternalInput") for n in range(BC)]
    te_d = [nc.dram_tensor(f"te{n}", [128, C, HR, HC], BF16,
                           kind="ExternalInput") for n in range(BC)]
    to_d = [nc.dram_tensor(f"to{n}", [128, C, HR, HC], BF16,
                           kind="ExternalInput") for n in range(BC)]
    out_d = nc.dram_tensor("out", [128, 1], F32, kind="ExternalOutput")

    # shift order: for each batch item, even-j shifts (aligned copy) first;
    # batch 0 fully before batch 1's odd-j so DMAs hide behind compute.
    phases = []
    for n in range(BC):
        phases.append((n, [(i, j) for i in range(K) for j in range(K)
                           if j % 2 == 0]))
    for n in range(BC):
        phases.append((n, [(i, j) for i in range(K) for j in range(K)
                           if j % 2 == 1]))

    with tile.TileContext(nc) as tc:
        with (
            tc.tile_pool(name="persist", bufs=1) as pp,
            tc.tile_pool(name="work", bufs=3) as wp,
        ):
            xt = [pp.tile([128, C, PR, PC], BF16, name=f"x{n}", tag=f"x{n}")
                  for n in range(BC)]
            te = [pp.tile([128, C, HR, HC], BF16, name=f"te{n}", tag=f"te{n}")
                  for n in range(BC)]
            to = [pp.tile([128, C, HR, HC], BF16, name=f"to{n}", tag=f"to{n}")
                  for n in range(BC)]
            best = [pp.tile([128, PR * PC], BF16, name=f"best{n}", tag=f"best{n}")
                    for n in range(BC)]

            # loads, in the order compute consumes them
            for n in range(BC):
                for c in range(C):
                    nc.sync.dma_start(out=xt[n][:, c], in_=x_d[n].ap()[:, c])
                for c in range(C):
                    nc.sync.dma_start(out=te[n][:, c], in_=te_d[n].ap()[:, c])
            for n in range(BC):
                for c in range(C):
                    nc.sync.dma_start(out=to[n][:, c], in_=to_d[n].ap()[:, c])

            rsum = pp.tile([128, 1], F32, tag="rsum")
            rtmp = pp.tile([128, 1], F32, tag="rtmp")
            seen = [0] * BC
            for pi, (n, shifts) in enumerate(phases):
                for i, j in shifts:
                    tsel, joff = (te[n], j) if j % 2 == 0 else (to[n], j - 1)
                    diff = wp.tile([128, C, PR, PC], BF16, tag="diff")
                    nc.vector.tensor_tensor(
                        out=diff[:], in0=xt[n][:],
                        in1=tsel[:, :, i:i + PR, joff:joff + PC],
                        op=mybir.AluOpType.subtract)
                    ab = wp.tile([128, C, PR, PC], BF16, tag="ab")
                    nc.scalar.activation(
                        out=ab[:], in_=diff[:],
                        func=mybir.ActivationFunctionType.Abs)
                    s1 = wp.tile([128, PR * PC], BF16, tag="s1")
                    a2 = ab[:].rearrange("p c r w -> p c (r w)")
                    nc.vector.tensor_tensor(
                        out=s1[:], in0=a2[:, 0], in1=a2[:, 1],
                        op=mybir.AluOpType.add)
                    if seen[n] == 0:
                        # first shift for this batch item: write straight in
                        nc.vector.tensor_tensor(
                            out=best[n][:], in0=s1[:], in1=a2[:, 2],
                            op=mybir.AluOpType.add)
                    else:
                        s2 = wp.tile([128, PR * PC], BF16, tag="s2")
                        nc.vector.tensor_tensor(
                            out=s2[:], in0=s1[:], in1=a2[:, 2],
                            op=mybir.AluOpType.add)
                        nc.vector.tensor_tensor(
                            out=best[n][:], in0=best[n][:], in1=s2[:],
                            op=mybir.AluOpType.min)
                    seen[n] += 1
                # once a batch item's min chain is complete, reduce it while
                # the other batch item's last phase still runs
                if pi == len(phases) - 2:
                    nc.vector.tensor_reduce(
                        out=rsum[:], in_=best[n][:], axis=mybir.AxisListType.X,
                        op=mybir.AluOpType.add)

            nc.vector.tensor_reduce(
                out=rtmp[:], in_=best[phases[-1][0]][:],
                axis=mybir.AxisListType.X, op=mybir.AluOpType.add)
            nc.vector.tensor_tensor(out=rsum[:], in0=rsum[:], in1=rtmp[:],
                                    op=mybir.AluOpType.add)
            nc.sync.dma_start(out=out_d.ap(), in_=rsum[:])

    nc.compile()
    return nc


def _prep(inputs, targets):
    bf = ml_dtypes.bfloat16
    inputs = np.asarray(inputs, dtype=np.float32)
    targets = np.asarray(targets, dtype=np.float32)
    x_bf = inputs.astype(bf)
    tp = np.zeros((B, C, HP, WP), dtype=np.float32)
    tp[:, :, PAD:PAD + H, PAD:PAD + W] = targets
    t_bf = tp.astype(bf)

    def halo(base):                       # base: [C, HP, >=WP-1] bf16 view
        s = base.strides
        v = np.lib.stride_tricks.as_strided(
            base, shape=(C, PH, PW, HR, HC),
            strides=(s[0], PR * s[1], PC * s[2], s[1], s[2]))
        return np.ascontiguousarray(v.transpose(1, 2, 0, 3, 4)
                                    .reshape(128, C, HR, HC))

    in_maps = []
    for core in range(NCORES):
        m = {}
        for n in range(BC):
            b = core * BC + n
            m[f"x{n}"] = np.ascontiguousarray(
                x_bf[b].reshape(C, PH, PR, PW, PC)
                       .transpose(1, 3, 0, 2, 4).reshape(128, C, PR, PC))
            m[f"te{n}"] = halo(t_bf[b])
            m[f"to{n}"] = halo(t_bf[b, :, :, 1:])
        in_maps.append(m)
    return in_maps


def _run(inputs, targets, trace=False, **kw):
    if "nc" not in _CACHE:
        _CACHE["nc"] = _build()
    nc = _CACHE["nc"]
    in_maps = _prep(inputs, targets)
    res = run_bass_kernel_spmd(nc, in_maps, list(range(NCORES)),
                               trace=trace, **kw)
    total = 0.0
    for core in range(NCORES):
        total += res.results[core]["out"].astype(np.float64).sum()
    val = np.float32(total / (C * B * H * W))
    return np.asarray(val, dtype=np.float32), res


def kernel(inputs, targets):
    out, _ = _run(inputs, targets)
    return out


# revision 15
# speedup vs baseline: 1.0259x; 1.0055x over previous
"""LocalL1Loss Trainium2 kernel (8 NeuronCores, pure data parallel).

Reference semantics (KERNEL_SIZE=7):
    tp = zero-pad(targets, 3 on each spatial side)
    d_s = mean_c |inputs - shift_s(tp)|      for the 49 shifts s
    out = mean_{n,h,w} min_s d_s

Per core (2 of 16 batch items):
  - host: cast to bf16; zero-pad targets; pre-arrange into the exact SBUF
    layouts (128 partitions = 16x8 grid of 32x64 pixel patches, 3 channel
    planes per batch item per partition; targets carry a 3px halo -> 38x70
    per patch, stored twice with a one-element column offset so every column
    shift reads a 4-byte-aligned bf16 window -> VectorE 2x mode throughout).
    Loads are a few large fully-contiguous DMAs.
  - per shift: VectorE subtract (bf16 2x), ScalarE Abs (parallel engine),
    VectorE 2 adds (channel sum) + running min.  Shifts are ordered so
    compute starts as soon as the first batch item's aligned-copy targets
    land; the remaining DMAs hide behind compute (min is order-invariant).
  - epilogue: free-dim reduce_sum -> [128,1] fp32 partials -> DRAM; host sums
    8x128 partials and divides by 3*N*H*W.
"""

import numpy as np
import ml_dtypes

import concourse.bacc as bacc
import concourse.mybir as mybir
from concourse import tile
from concourse.bass_utils import run_bass_kernel_spmd

# geometry (hardcoded for the [16, 3, 512, 512] problem)
B, C, H, W = 16, 3, 512, 512
K = 7
PAD = K // 2
NCORES = 8
BC = B // NCORES            # batch per core = 2
PH, PW = 16, 8              # patch grid -> 128 partitions
PR, PC = H // PH, W // PW   # 32 x 64 patch
HR, HC = PR + K - 1, PC + K - 1          # 38 x 70 halo patch
HP, WP = H + K - 1, W + K - 1 + 1        # padded target: 518 x 519 (+1 col)

BF16 = mybir.dt.bfloat16
F32 = mybir.dt.float32

_CACHE = {}


def _build():
    nc = bacc.Bacc("TRN2", target_bir_lowering=False, debug=False,
                   num_devices=NCORES)

    x_d = [nc.dram_tensor(f"x{n}", [128, C, PR, PC], BF16,
                          kind="ExternalInput") for n in range(BC)]
    te_d = [nc.dram_tensor(f"te{n}", [128, C, HR, HC], BF16,
                           kind="ExternalInput") for n in range(BC)]
    to_d = [nc.dram_tensor(f"to{n}", [128, C, HR, HC], BF16,
                           kind="ExternalInput") for n in range(BC)]
    out_d = nc.dram_tensor("out", [128, 1], F32, kind="ExternalOutput")

    # shift order: for each batch item, even-j shifts (aligned copy) first;
    # batch 0 fully before batch 1's odd-j so DMAs hide behind compute.
    phases = []
    for n in range(BC):
        phases.append((n, [(i, j) for i in range(K) for j in range(K)
                           if j % 2 == 0]))
    for n in range(BC):
        phases.append((n, [(i, j) for i in range(K) for j in range(K)
                           if j % 2 == 1]))

    with tile.TileContext(nc) as tc:
        with (
            tc.tile_pool(name="persist", bufs=1) as pp,
            tc.tile_pool(name="work", bufs=3) as wp,
        ):
            xt = [pp.tile([128, C, PR, PC], BF16, name=f"x{n}", tag=f"x{n}")
                  for n in range(BC)]
            te = [pp.tile([128, C, HR, HC], BF16, name=f"te{n}", tag=f"te{n}")
                  for n in range(BC)]
            to = [pp.tile([128, C, HR, HC], BF16, name=f"to{n}", tag=f"to{n}")
                  for n in range(BC)]
            best = [pp.tile([128, PR * PC], BF16, name=f"best{n}", tag=f"best{n}")
                    for n in range(BC)]

            # loads, in the order compute consumes them: channel-interleaved
            # so the first per-channel subtract can start after ~1.2 MB
            for n in range(BC):
                for c in range(C):
                    nc.sync.dma_start(out=xt[n][:, c], in_=x_d[n].ap()[:, c])
                    nc.sync.dma_start(out=te[n][:, c], in_=te_d[n].ap()[:, c])
            for n in range(BC):
                for c in range(C):
                    nc.sync.dma_start(out=to[n][:, c], in_=to_d[n].ap()[:, c])

            rsum = pp.tile([128, 1], F32, tag="rsum")
            rtmp = pp.tile([128, 1], F32, tag="rtmp")
            seen = [0] * BC
            for pi, (n, shifts) in enumerate(phases):
                for si, (i, j) in enumerate(shifts):
                    tsel, joff = (te[n], j) if j % 2 == 0 else (to[n], j - 1)
                    diff = wp.tile([128, C, PR, PC], BF16, tag="diff")
                    if pi == 0 and si == 0:
                        # per-channel subs so compute starts after the first
                        # (x, te) channel pair lands, not the whole tiles
                        for c in range(C):
                            nc.vector.tensor_tensor(
                                out=diff[:, c], in0=xt[n][:, c],
                                in1=tsel[:, c, i:i + PR, joff:joff + PC],
                                op=mybir.AluOpType.subtract)
                    else:
                        nc.vector.tensor_tensor(
                            out=diff[:], in0=xt[n][:],
                            in1=tsel[:, :, i:i + PR, joff:joff + PC],
                            op=mybir.AluOpType.subtract)
                    ab = wp.tile([128, C, PR, PC], BF16, tag="ab")
                    nc.scalar.activation(
                        out=ab[:], in_=diff[:],
                        func=mybir.ActivationFunctionType.Abs)
                    # during the last shift's Abs wait, fold in the finished
                    # batch item's partial-sum reduce
                    if pi == len(phases) - 1 and si == len(shifts) - 1:
                        other = phases[-2][0]
                        nc.vector.tensor_reduce(
                            out=rsum[:], in_=best[other][:],
                            axis=mybir.AxisListType.X, op=mybir.AluOpType.add)
                    s1 = wp.tile([128, PR * PC], BF16, tag="s1")
                    a2 = ab[:].rearrange("p c r w -> p c (r w)")
                    nc.vector.tensor_tensor(
                        out=s1[:], in0=a2[:, 0], in1=a2[:, 1],
                        op=mybir.AluOpType.add)
                    if seen[n] == 0:
                        # first shift for this batch item: write straight in
                        nc.vector.tensor_tensor(
                            out=best[n][:], in0=s1[:], in1=a2[:, 2],
                            op=mybir.AluOpType.add)
                    else:
                        s2 = wp.tile([128, PR * PC], BF16, tag="s2")
                        nc.vector.tensor_tensor(
                            out=s2[:], in0=s1[:], in1=a2[:, 2],
                            op=mybir.AluOpType.add)
                        nc.vector.tensor_tensor(
                            out=best[n][:], in0=best[n][:], in1=s2[:],
                            op=mybir.AluOpType.min)
                    seen[n] += 1

            nc.vector.tensor_reduce(
                out=rtmp[:], in_=best[phases[-1][0]][:],
                axis=mybir.AxisListType.X, op=mybir.AluOpType.add)
            nc.vector.tensor_tensor(out=rsum[:], in0=rsum[:], in1=rtmp[:],
                                    op=mybir.AluOpType.add)
            nc.sync.dma_start(out=out_d.ap(), in_=rsum[:])

    nc.compile()
    return nc


def _prep(inputs, targets):
    bf = ml_dtypes.bfloat16
    inputs = np.asarray(inputs, dtype=np.float32)
    targets = np.asarray(targets, dtype=np.float32)
    x_bf = inputs.astype(bf)
    tp = np.zeros((B, C, HP, WP), dtype=np.float32)
    tp[:, :, PAD:PAD + H, PAD:PAD + W] = targets
    t_bf = tp.astype(bf)

    def halo(base):                       # base: [C, HP, >=WP-1] bf16 view
        s = base.strides
        v = np.lib.stride_tricks.as_strided(
            base, shape=(C, PH, PW, HR, HC),
            strides=(s[0], PR * s[1], PC * s[2], s[1], s[2]))
        return np.ascontiguousarray(v.transpose(1, 2, 0, 3, 4)
                                    .reshape(128, C, HR, HC))

    in_maps = []
    for core in range(NCORES):
        m = {}
        for n in range(BC):
            b = core * BC + n
            m[f"x{n}"] = np.ascontiguousarray(
                x_bf[b].reshape(C, PH, PR, PW, PC)
                       .transpose(1, 3, 0, 2, 4).reshape(128, C, PR, PC))
            m[f"te{n}"] = halo(t_bf[b])
            m[f"to{n}"] = halo(t_bf[b, :, :, 1:])
        in_maps.append(m)
    return in_maps


def _run(inputs, targets, trace=False, **kw):
    if "nc" not in _CACHE:
        _CACHE["nc"] = _build()
    nc = _CACHE["nc"]
    in_maps = _prep(inputs, targets)
    res = run_bass_kernel_spmd(nc, in_maps, list(range(NCORES)),
                               trace=trace, **kw)
    total = 0.0
    for core in range(NCORES):
        total += res.results[core]["out"].astype(np.float64).sum()
    val = np.float32(total / (C * B * H * W))
    return np.asarray(val, dtype=np.float32), res


def kernel(inputs, targets):
    out, _ = _run(inputs, targets)
    return out


# revision 19
# speedup vs baseline: 1.0356x; 1.0094x over previous
"""LocalL1Loss Trainium2 kernel (8 NeuronCores, pure data parallel).

Reference semantics (KERNEL_SIZE=7):
    tp = zero-pad(targets, 3 on each spatial side)
    d_s = mean_c |inputs - shift_s(tp)|      for the 49 shifts s
    out = mean_{n,h,w} min_s d_s

Per core (2 of 16 batch items):
  - host: cast to bf16; zero-pad targets; pre-arrange into the exact SBUF
    layouts (128 partitions = 16x8 grid of 32x64 pixel patches, 3 channel
    planes per batch item per partition; targets carry a 3px halo -> 38x70
    per patch, stored twice with a one-element column offset so every column
    shift reads a 4-byte-aligned bf16 window -> VectorE 2x mode throughout).
    Loads are a few large fully-contiguous DMAs.
  - per shift: VectorE subtract (bf16 2x), ScalarE Abs (parallel engine),
    VectorE 2 adds (channel sum) + running min.  Shifts are ordered so
    compute starts as soon as the first batch item's aligned-copy targets
    land; the remaining DMAs hide behind compute (min is order-invariant).
  - epilogue: free-dim reduce_sum -> [128,1] fp32 partials -> DRAM; host sums
    8x128 partials and divides by 3*N*H*W.
"""

import numpy as np
import ml_dtypes

import concourse.bacc as bacc
import concourse.mybir as mybir
from concourse import tile
from concourse.bass_utils import run_bass_kernel_spmd

# geometry (hardcoded for the [16, 3, 512, 512] problem)
B, C, H, W = 16, 3, 512, 512
K = 7
PAD = K // 2
NCORES = 8
BC = B // NCORES            # batch per core = 2
PH, PW = 16, 8              # patch grid -> 128 partitions
PR, PC = H // PH, W // PW   # 32 x 64 patch
HR, HC = PR + K - 1, PC + K - 1          # 38 x 70 halo patch
HP, WP = H + K - 1, W + K - 1 + 1        # padded target: 518 x 519 (+1 col)

BF16 = mybir.dt.bfloat16
F32 = mybir.dt.float32

_CACHE = {}


def _build():
    nc = bacc.Bacc("TRN2", target_bir_lowering=False, debug=False,
                   num_devices=NCORES)

    x_d = [nc.dram_tensor(f"x{n}", [128, C, PR, PC], BF16,
                          kind="ExternalInput") for n in range(BC)]
    te_d = [nc.dram_tensor(f"te{n}", [128, C, HR, HC], BF16,
                           kind="ExternalInput") for n in range(BC)]
    to_d = [nc.dram_tensor(f"to{n}", [128, C, HR, HC], BF16,
                           kind="ExternalInput") for n in range(BC)]
    out_d = nc.dram_tensor("out", [1, 1], F32, kind="ExternalOutput")

    # shift order: for each batch item, even-j shifts (aligned copy) first;
    # batch 0 fully before batch 1's odd-j so DMAs hide behind compute.
    phases = []
    for n in range(BC):
        phases.append((n, [(i, j) for i in range(K) for j in range(K)
                           if j % 2 == 0]))
    for n in range(BC):
        phases.append((n, [(i, j) for i in range(K) for j in range(K)
                           if j % 2 == 1]))

    with tile.TileContext(nc) as tc:
        with (
            tc.tile_pool(name="persist", bufs=1) as pp,
            tc.tile_pool(name="work", bufs=3) as wp,
            tc.tile_pool(name="psum", bufs=1, space="PSUM") as qp,
        ):
            xt = [pp.tile([128, C, PR, PC], BF16, name=f"x{n}", tag=f"x{n}")
                  for n in range(BC)]
            te = [pp.tile([128, C, HR, HC], BF16, name=f"te{n}", tag=f"te{n}")
                  for n in range(BC)]
            to = [pp.tile([128, C, HR, HC], BF16, name=f"to{n}", tag=f"to{n}")
                  for n in range(BC)]
            best = [pp.tile([128, PR * PC], BF16, name=f"best{n}", tag=f"best{n}")
                    for n in range(BC)]

            # loads, in the order compute consumes them: channel-interleaved
            # so the first per-channel subtract can start after ~1.2 MB
            for n in range(BC):
                for c in range(C):
                    nc.sync.dma_start(out=xt[n][:, c], in_=x_d[n].ap()[:, c])
                    nc.sync.dma_start(out=te[n][:, c], in_=te_d[n].ap()[:, c])
            for n in range(BC):
                for c in range(C):
                    nc.sync.dma_start(out=to[n][:, c], in_=to_d[n].ap()[:, c])

            rsum = pp.tile([128, 1], F32, tag="rsum")
            rtmp = pp.tile([128, 1], F32, tag="rtmp")
            ones = pp.tile([128, 1], F32, name="ones", tag="ones")
            nc.vector.memset(ones[:], 1.0)
            seen = [0] * BC
            for pi, (n, shifts) in enumerate(phases):
                for si, (i, j) in enumerate(shifts):
                    tsel, joff = (te[n], j) if j % 2 == 0 else (to[n], j - 1)
                    diff = wp.tile([128, C, PR, PC], BF16, tag="diff")
                    if pi == 0 and si == 0:
                        # per-channel subs so compute starts after the first
                        # (x, te) channel pair lands, not the whole tiles
                        for c in range(C):
                            nc.vector.tensor_tensor(
                                out=diff[:, c], in0=xt[n][:, c],
                                in1=tsel[:, c, i:i + PR, joff:joff + PC],
                                op=mybir.AluOpType.subtract)
                    else:
                        nc.vector.tensor_tensor(
                            out=diff[:], in0=xt[n][:],
                            in1=tsel[:, :, i:i + PR, joff:joff + PC],
                            op=mybir.AluOpType.subtract)
                    ab = wp.tile([128, C, PR, PC], BF16, tag="ab")
                    nc.scalar.activation(
                        out=ab[:], in_=diff[:],
                        func=mybir.ActivationFunctionType.Abs)
                    # during the last shift's Abs wait, fold in the finished
                    # batch item's partial-sum reduce
                    if pi == len(phases) - 1 and si == len(shifts) - 1:
                        other = phases[-2][0]
                        nc.vector.tensor_reduce(
                            out=rsum[:], in_=best[other][:],
                            axis=mybir.AxisListType.X, op=mybir.AluOpType.add)
                    s1 = wp.tile([128, PR * PC], BF16, tag="s1")
                    a2 = ab[:].rearrange("p c r w -> p c (r w)")
                    nc.vector.tensor_tensor(
                        out=s1[:], in0=a2[:, 0], in1=a2[:, 1],
                        op=mybir.AluOpType.add)
                    if seen[n] == 0:
                        # first shift for this batch item: write straight in
                        nc.vector.tensor_tensor(
                            out=best[n][:], in0=s1[:], in1=a2[:, 2],
                            op=mybir.AluOpType.add)
                    else:
                        s2 = wp.tile([128, PR * PC], BF16, tag="s2")
                        nc.vector.tensor_tensor(
                            out=s2[:], in0=s1[:], in1=a2[:, 2],
                            op=mybir.AluOpType.add)
                        nc.vector.tensor_tensor(
                            out=best[n][:], in0=best[n][:], in1=s2[:],
                            op=mybir.AluOpType.min)
                    seen[n] += 1

            nc.vector.tensor_reduce(
                out=rtmp[:], in_=best[phases[-1][0]][:],
                axis=mybir.AxisListType.X, op=mybir.AluOpType.add)
            nc.vector.tensor_tensor(out=rsum[:], in0=rsum[:], in1=rtmp[:],
                                    op=mybir.AluOpType.add)
            # partition-reduce [128,1] -> scalar on TensorE so the output
            # DMA is a single descriptor, not 128 four-byte ones (~7us)
            psc = qp.tile([1, 1], F32, name="psc", tag="psc")
            nc.tensor.matmul(psc[:], lhsT=ones[:], rhs=rsum[:],
                             start=True, stop=True)
            osc = pp.tile([1, 1], F32, name="osc", tag="osc")
            nc.scalar.copy(out=osc[:], in_=psc[:])
            nc.sync.dma_start(out=out_d.ap(), in_=osc[:])

    nc.compile()
    return nc


def _prep(inputs, targets):
    bf = ml_dtypes.bfloat16
    inputs = np.asarray(inputs, dtype=np.float32)
    targets = np.asarray(targets, dtype=np.float32)
    x_bf = inputs.astype(bf)
    tp = np.zeros((B, C, HP, WP), dtype=np.float32)
    tp[:, :, PAD:PAD + H, PAD:PAD + W] = targets
    t_bf = tp.astype(bf)

    def halo(base):                       # base: [C, HP, >=WP-1] bf16 view
        s = base.strides
        v = np.lib.stride_tricks.as_strided(
            base, shape=(C, PH, PW, HR, HC),
            strides=(s[0], PR * s[1], PC * s[2], s[1], s[2]))
        return np.ascontiguousarray(v.transpose(1, 2, 0, 3, 4)
                                    .reshape(128, C, HR, HC))

    in_maps = []
    for core in range(NCORES):
        m = {}
        for n in range(BC):
            b = core * BC + n
            m[f"x{n}"] = np.ascontiguousarray(
                x_bf[b].reshape(C, PH, PR, PW, PC)
                       .transpose(1, 3, 0, 2, 4).reshape(128, C, PR, PC))
            m[f"te{n}"] = halo(t_bf[b])
            m[f"to{n}"] = halo(t_bf[b, :, :, 1:])
        in_maps.append(m)
    return in_maps


def _run(inputs, targets, trace=False, **kw):
    if "nc" not in _CACHE:
        _CACHE["nc"] = _build()
    nc = _CACHE["nc"]
    in_maps = _prep(inputs, targets)
    res = run_bass_kernel_spmd(nc, in_maps, list(range(NCORES)),
                               trace=trace, **kw)
    total = 0.0
    for core in range(NCORES):
        total += res.results[core]["out"].astype(np.float64).sum()
    val = np.float32(total / (C * B * H * W))
    return np.asarray(val, dtype=np.float32), res


def kernel(inputs, targets):
    out, _ = _run(inputs, targets)
    return out
